# revision 6
# baseline (speedup 1.0000x reference)
"""Bahdanau attention with coverage — distributed Bass kernel for 8 TRN2 NeuronCores.

Data-parallel over batch: each core handles B/8 = 4 examples end-to-end; the
small projection weights are replicated. No inter-core collectives needed.

Per example (T=2048, DQ=DV=UNITS=1024):
  H[t,u]   = values[t,:] @ Ws[:,u] + cov[t]*Wc[u] + (query @ Wh + bh + bs + bc)[u]
  score[t] = tanh(H[t,:]) @ Vw + Vb
  attn     = softmax(score) (no max-shift needed: |score| stays tiny)
  coverage = attn + prev_coverage
  context  = attn @ values

On-chip layout: H is computed transposed ([u_partitions, t_free]) so the
query/bias terms become per-partition activation biases, the coverage term a
rank-1 (K=1) PSUM-accumulated matmul, and score a PE contraction over u.
values tiles are PE-transposed on the fly for the main matmul; the same native
tiles feed the context matmul, with unnormalized exp weights accumulated in
PSUM across chunks and one rescale at the end. Matmuls run in float32r
(~TF32 precision, 4x the fp32 PE throughput at moving dim >= 256).

Scheduling notes:
  - matmul start=True zeroes the whole 2KB PSUM bank ("zero region"), so only
    the first matmul/transpose into a bank sets it; later ones accumulate.
  - the score matmul of u-tile uo is emitted after the main matmuls of uo+1,
    and the context matmuls of chunk ch after the transposes of chunk ch+1,
    so the PE never stalls waiting on ACT.
"""

import numpy as np

B, T, DQ, DV, UNITS = 32, 2048, 1024, 1024, 1024
NC_ = 8
BL = B // NC_          # 4 examples per core
P = 128
TCH = 512              # t-chunk size (main-MM moving dim)
NCH = T // TCH         # 4 chunks per example
TT = TCH // P          # 4 t-tiles per chunk
KO = DV // P           # 8 contraction chunks
UO = UNITS // P        # 8 u tiles
DO = DV // 512         # 2 dv chunks for context MM

_cache = {}


def _build():
    import concourse.mybir as mybir
    from concourse import bacc
    import concourse.tile as tile
    from concourse.masks import make_identity

    f32 = mybir.dt.float32
    f32r = mybir.dt.float32r
    Tanh = mybir.ActivationFunctionType.Tanh
    Exp = mybir.ActivationFunctionType.Exp
    Add = mybir.AluOpType.add
    X = mybir.AxisListType.X

    nc = bacc.Bacc('TRN2', target_bir_lowering=False, debug=False, num_devices=NC_)
    q_d = nc.dram_tensor('query', [BL, DQ], f32, kind='ExternalInput').ap()
    v_d = nc.dram_tensor('values', [BL, T, DV], f32, kind='ExternalInput').ap()
    c_d = nc.dram_tensor('prev_coverage', [BL, T], f32, kind='ExternalInput').ap()
    wh_d = nc.dram_tensor('Wh', [DQ, UNITS], f32, kind='ExternalInput').ap()
    bh_d = nc.dram_tensor('bh', [UNITS], f32, kind='ExternalInput').ap()
    ws_d = nc.dram_tensor('Ws', [DV, UNITS], f32, kind='ExternalInput').ap()
    bs_d = nc.dram_tensor('bs', [UNITS], f32, kind='ExternalInput').ap()
    wc_d = nc.dram_tensor('Wc', [1, UNITS], f32, kind='ExternalInput').ap()
    bc_d = nc.dram_tensor('bc', [UNITS], f32, kind='ExternalInput').ap()
    vw_d = nc.dram_tensor('Vw', [UNITS], f32, kind='ExternalInput').ap()
    vb_d = nc.dram_tensor('Vb', [1, 1], f32, kind='ExternalInput').ap()
    ctx_d = nc.dram_tensor('ctx', [BL, DV], f32, kind='ExternalOutput').ap()
    attn_d = nc.dram_tensor('attn', [BL, T], f32, kind='ExternalOutput').ap()
    cov_d = nc.dram_tensor('cov', [BL, T], f32, kind='ExternalOutput').ap()

    with tile.TileContext(nc) as tc:
        with tc.tile_pool(name='const', bufs=1) as cpool, \
             tc.tile_pool(name='ptr', bufs=3, space='PSUM') as p_tr, \
             tc.tile_pool(name='ph', bufs=2, space='PSUM') as p_h, \
             tc.tile_pool(name='psc', bufs=1, space='PSUM') as p_sc, \
             tc.tile_pool(name='pctx', bufs=1, space='PSUM') as p_ctx:

            # ---- persistent constants ----
            ws_sb = cpool.tile([P, KO, UNITS], f32r, tag='ws')
            nc.gpsimd.dma_start(ws_sb[:], ws_d.rearrange("(ko kp) u -> kp ko u", kp=P))
            wc_sb = cpool.tile([1, UNITS], f32r, tag='wc')
            nc.gpsimd.dma_start(wc_sb[:], wc_d[:, :])
            vw_sb = cpool.tile([P, UO], f32r, tag='vw')
            nc.gpsimd.dma_start(vw_sb[:], vw_d.rearrange("(o p) -> p o", p=P))
            vb_sb = cpool.tile([1, 1], f32, tag='vb')
            nc.sync.dma_start(vb_sb[:], vb_d[:, :])
            id_r = cpool.tile([P, P], f32r, tag='idr')
            ones1 = cpool.tile([1, 1], f32, tag='one')
            nc.vector.memset(ones1[:], 1.0)
            qbias = cpool.tile([P, UO, BL], f32, tag='qb')

            # ---- prologue (scoped pool: space reclaimed before the main loop) ----
            with tc.tile_pool(name='prolog', bufs=1) as ppool:
                wh_sb = ppool.tile([P, KO, UNITS], f32, tag='wh')
                nc.sync.dma_start(wh_sb[:], wh_d.rearrange("(ko kp) u -> kp ko u", kp=P))

                bias3 = ppool.tile([P, UO], f32, tag='b3')
                btmp1 = ppool.tile([P, UO], f32, tag='bt1')
                btmp2 = ppool.tile([P, UO], f32, tag='bt2')
                nc.sync.dma_start(bias3[:], bh_d.rearrange("(o p) -> p o", p=P))
                nc.sync.dma_start(btmp1[:], bs_d.rearrange("(o p) -> p o", p=P))
                nc.sync.dma_start(btmp2[:], bc_d.rearrange("(o p) -> p o", p=P))
                nc.vector.tensor_add(bias3[:], bias3[:], btmp1[:])
                nc.vector.tensor_add(bias3[:], bias3[:], btmp2[:])

                id_s = ppool.tile([P, P], f32, tag='id')
                make_identity(nc, id_s[:])
                nc.vector.tensor_copy(id_r[:], id_s[:])

                # query -> qT [dq_p, dq_o, BL] via PE transposes
                q_nat = ppool.tile([BL, DQ], f32, tag='qn')
                nc.sync.dma_start(q_nat[:], q_d[:, :])
                qT = ppool.tile([P, KO, BL], f32, tag='qT')
                for ko in range(KO):
                    pq = p_tr.tile([P, BL], f32, tag='tr')
                    nc.tensor.transpose(pq[:], q_nat[:, ko * P:(ko + 1) * P], id_s[0:BL, 0:BL])
                    nc.vector.tensor_copy(qT[:, ko, :], pq[:])

                # qbias[u, uo, b] = (query @ Wh)[b, u] + bh[u] + bs[u] + bc[u]
                for uo in range(UO):
                    pqp = p_tr.tile([P, BL], f32, tag='tr')
                    for ko in range(KO):
                        nc.tensor.matmul(pqp[:], wh_sb[:, ko, uo * P:(uo + 1) * P], qT[:, ko, :],
                                         start=(ko == 0), stop=(ko == KO - 1))
                    nc.vector.tensor_tensor(qbias[:, uo, :], pqp[:],
                                            bias3[:, uo:uo + 1].to_broadcast([P, BL]), Add)

            # ---- main loop over examples and t-chunks ----
            with tc.tile_pool(name='val', bufs=3) as vpool, \
                 tc.tile_pool(name='vt', bufs=2) as vtpool, \
                 tc.tile_pool(name='th', bufs=3) as hpool, \
                 tc.tile_pool(name='small', bufs=2) as spool:

                def emit_ctx(ch, vn, exp_b, pctx):
                    # transpose exp row into [t_p, tt], then accumulate
                    # pctx[0, d] += sum_t exp[t] * values[t, d]
                    petr = p_tr.tile([P, TT], f32, tag='tr')
                    for tt in range(TT):
                        nc.tensor.matmul(petr[:, tt:tt + 1],
                                         exp_b[0:1, (ch * TT + tt) * P:(ch * TT + tt + 1) * P],
                                         ones1[:], is_transpose=True,
                                         start=(tt == 0), stop=(tt == TT - 1),
                                         skip_group_check=True)
                    eT = spool.tile([P, TT], f32r, tag='eT')
                    nc.vector.tensor_copy(eT[:], petr[:])
                    for do in range(DO):
                        for tt in range(TT):
                            nc.tensor.matmul(pctx[0:1, do * 512:(do + 1) * 512],
                                             eT[:, tt:tt + 1], vn[:, tt, do * 512:(do + 1) * 512],
                                             start=(ch == 0 and tt == 0),
                                             stop=(ch == NCH - 1 and tt == TT - 1),
                                             skip_group_check=True)

                for b in range(BL):
                    exp_b = spool.tile([1, T], f32, tag='exp')
                    acc_b = spool.tile([1, NCH], f32, tag='acc')
                    cov_f = spool.tile([1, T], f32, tag='cvf')
                    nc.sync.dma_start(cov_f[:], c_d[b:b + 1, :])
                    pctx = p_ctx.tile([1, DV], f32, tag='ctx')
                    vns = []
                    for ch in range(NCH):
                        # native values chunk [tp, tt, d], rounded to f32r by the DMA
                        vn = vpool.tile([P, TT, DV], f32r, tag='vn')
                        nc.gpsimd.dma_start(
                            vn[:], v_d[b].rearrange("(tt tp) d -> tp tt d", tp=P)[:, ch * TT:(ch + 1) * TT, :])
                        cvr = spool.tile([1, TCH], f32r, tag='cvr')
                        nc.gpsimd.dma_start(cvr[:], c_d[b:b + 1, ch * TCH:(ch + 1) * TCH])

                        # transpose to [dv_p, ko, t] for the main matmul
                        vt = vtpool.tile([P, KO, TCH], f32r, tag='vt')
                        for ko in range(KO):
                            ptr = p_tr.tile([P, TCH], f32r, tag='tr')
                            for tt in range(TT):
                                nc.tensor.matmul(ptr[:, tt * P:(tt + 1) * P],
                                                 vn[:, tt, ko * P:(ko + 1) * P], id_r[:],
                                                 is_transpose=True,
                                                 start=(tt == 0), stop=(tt == TT - 1),
                                                 skip_group_check=True)
                            nc.vector.tensor_copy(vt[:, ko, :], ptr[:].bitcast(f32))

                        # H^T tiles + tanh + score contraction over u
                        psc = p_sc.tile([1, TCH], f32, tag='sc')
                        ths = []
                        for uo in range(UO):
                            ph = p_h.tile([P, TCH], f32, tag='h')
                            nc.tensor.matmul(ph[:], wc_sb[0:1, uo * P:(uo + 1) * P], cvr[:],
                                             start=True, stop=False)
                            for ko in range(KO):
                                nc.tensor.matmul(ph[:], ws_sb[:, ko, uo * P:(uo + 1) * P], vt[:, ko, :],
                                                 start=False, stop=(ko == KO - 1))
                            th = hpool.tile([P, TCH], f32r, tag='th')
                            nc.scalar.activation(th[:], ph[:], Tanh, bias=qbias[:, uo, b:b + 1])
                            ths.append(th)
                            if uo > 0:
                                nc.tensor.matmul(psc[:], vw_sb[:, uo - 1:uo], ths[uo - 1][:],
                                                 start=(uo == 1), stop=False,
                                                 skip_group_check=True)
                        nc.tensor.matmul(psc[:], vw_sb[:, UO - 1:UO], ths[UO - 1][:],
                                         start=False, stop=True, skip_group_check=True)

                        # exp (+ Vb bias) with free per-chunk sum
                        nc.scalar.activation(exp_b[0:1, ch * TCH:(ch + 1) * TCH], psc[:], Exp,
                                             bias=vb_sb[:], accum_out=acc_b[0:1, ch:ch + 1])

                        # context contribution of the PREVIOUS chunk (deferred
                        # one chunk so the PE rides ahead of ACT's exp)
                        if ch > 0:
                            emit_ctx(ch - 1, vns[ch - 1], exp_b, pctx)
                        vns.append(vn)
                    emit_ctx(NCH - 1, vns[NCH - 1], exp_b, pctx)

                    # ---- per-example epilogue ----
                    sum_b = spool.tile([1, 1], f32, tag='sum')
                    nc.vector.tensor_reduce(sum_b[:], acc_b[:], X, Add)
                    rec_b = spool.tile([1, 1], f32, tag='rec')
                    nc.vector.reciprocal(rec_b[:], sum_b[:])
                    ctx_sb = spool.tile([1, DV], f32, tag='ctxo')
                    nc.vector.tensor_scalar_mul(ctx_sb[:], pctx[:], rec_b[:])
                    nc.sync.dma_start(ctx_d[b:b + 1, :], ctx_sb[:])
                    nc.vector.tensor_scalar_mul(exp_b[:], exp_b[:], rec_b[:])
                    nc.sync.dma_start(attn_d[b:b + 1, :], exp_b[:])
                    nc.vector.tensor_add(cov_f[:], cov_f[:], exp_b[:])
                    nc.sync.dma_start(cov_d[b:b + 1, :], cov_f[:])

    nc.compile()
    return nc


def kernel(query, values, prev_coverage, Wh, bh, Ws, bs, Wc, bc, Vw, Vb):
    from concourse.bass_utils import run_bass_kernel_spmd

    if 'nc' not in _cache:
        _cache['nc'] = _build()
    nc = _cache['nc']

    query = np.ascontiguousarray(np.asarray(query, dtype=np.float32))
    values = np.ascontiguousarray(np.asarray(values, dtype=np.float32))
    cov = np.ascontiguousarray(
        np.asarray(prev_coverage, dtype=np.float32).reshape(B, T))
    rep = {
        'Wh': np.ascontiguousarray(np.asarray(Wh, dtype=np.float32)),
        'bh': np.ascontiguousarray(np.asarray(bh, dtype=np.float32)),
        'Ws': np.ascontiguousarray(np.asarray(Ws, dtype=np.float32)),
        'bs': np.ascontiguousarray(np.asarray(bs, dtype=np.float32)),
        'Wc': np.ascontiguousarray(np.asarray(Wc, dtype=np.float32).reshape(1, UNITS)),
        'bc': np.ascontiguousarray(np.asarray(bc, dtype=np.float32)),
        'Vw': np.ascontiguousarray(np.asarray(Vw, dtype=np.float32).reshape(UNITS)),
        'Vb': np.ascontiguousarray(np.asarray(Vb, dtype=np.float32).reshape(1, 1)),
    }
    in_maps = []
    for c in range(NC_):
        s = slice(c * BL, (c + 1) * BL)
        in_maps.append({
            'query': query[s],
            'values': values[s],
            'prev_coverage': cov[s],
            **rep,
        })

    res = run_bass_kernel_spmd(nc, in_maps, core_ids=list(range(NC_)))
    context = np.concatenate([res.results[c]['ctx'] for c in range(NC_)], axis=0)
    attn = np.concatenate([res.results[c]['attn'] for c in range(NC_)], axis=0)
    coverage = np.concatenate([res.results[c]['cov'] for c in range(NC_)], axis=0)
    return context, attn, coverage.reshape(B, T, 1)


# revision 9
# speedup vs baseline: 1.0138x; 1.0138x over previous
"""Bahdanau attention with coverage — distributed Bass kernel for 8 TRN2 NeuronCores.

Data-parallel over batch: each core handles B/8 = 4 examples end-to-end; the
small projection weights are replicated. No inter-core collectives needed.

Per example (T=2048, DQ=DV=UNITS=1024):
  H[t,u]   = values[t,:] @ Ws[:,u] + cov[t]*Wc[u] + (query @ Wh + bh + bs + bc)[u]
  score[t] = tanh(H[t,:]) @ Vw + Vb
  attn     = softmax(score) (no max-shift needed: |score| stays tiny)
  coverage = attn + prev_coverage
  context  = attn @ values

On-chip layout: H is computed transposed ([u_partitions, t_free]) so the
query/bias terms become per-partition activation biases, the coverage term a
rank-1 (K=1) PSUM-accumulated matmul, and score a PE contraction over u.
values tiles are PE-transposed on the fly for the main matmul; the same native
tiles feed the context matmul, with unnormalized exp weights accumulated in
PSUM across chunks and one rescale at the end. Matmuls run in float32r
(~TF32 precision, 4x the fp32 PE throughput at moving dim >= 256).

Scheduling notes:
  - matmul start=True zeroes the whole 2KB PSUM bank ("zero region"), so only
    the first matmul/transpose into a bank sets it; later ones accumulate.
  - the score matmul of u-tile uo is emitted after the main matmuls of uo+1,
    and the context matmuls of chunk ch after the transposes of chunk ch+1,
    so the PE never stalls waiting on ACT.
"""

import numpy as np

B, T, DQ, DV, UNITS = 32, 2048, 1024, 1024, 1024
NC_ = 8
BL = B // NC_          # 4 examples per core
P = 128
TCH = 512              # t-chunk size (main-MM moving dim)
NCH = T // TCH         # 4 chunks per example
TT = TCH // P          # 4 t-tiles per chunk
KO = DV // P           # 8 contraction chunks
UO = UNITS // P        # 8 u tiles
DO = DV // 512         # 2 dv chunks for context MM

_cache = {}


def _build():
    import concourse.mybir as mybir
    from concourse import bacc
    import concourse.tile as tile
    from concourse.masks import make_identity

    f32 = mybir.dt.float32
    f32r = mybir.dt.float32r
    Tanh = mybir.ActivationFunctionType.Tanh
    Exp = mybir.ActivationFunctionType.Exp
    Add = mybir.AluOpType.add
    X = mybir.AxisListType.X

    nc = bacc.Bacc('TRN2', target_bir_lowering=False, debug=False, num_devices=NC_)
    q_d = nc.dram_tensor('query', [BL, DQ], f32, kind='ExternalInput').ap()
    v_d = nc.dram_tensor('values', [BL, T, DV], f32, kind='ExternalInput').ap()
    c_d = nc.dram_tensor('prev_coverage', [BL, T], f32, kind='ExternalInput').ap()
    wh_d = nc.dram_tensor('Wh', [DQ, UNITS], f32, kind='ExternalInput').ap()
    bh_d = nc.dram_tensor('bh', [UNITS], f32, kind='ExternalInput').ap()
    ws_d = nc.dram_tensor('Ws', [DV, UNITS], f32, kind='ExternalInput').ap()
    bs_d = nc.dram_tensor('bs', [UNITS], f32, kind='ExternalInput').ap()
    wc_d = nc.dram_tensor('Wc', [1, UNITS], f32, kind='ExternalInput').ap()
    bc_d = nc.dram_tensor('bc', [UNITS], f32, kind='ExternalInput').ap()
    vw_d = nc.dram_tensor('Vw', [UNITS], f32, kind='ExternalInput').ap()
    vb_d = nc.dram_tensor('Vb', [1, 1], f32, kind='ExternalInput').ap()
    ctx_d = nc.dram_tensor('ctx', [BL, DV], f32, kind='ExternalOutput').ap()
    attn_d = nc.dram_tensor('attn', [BL, T], f32, kind='ExternalOutput').ap()
    cov_d = nc.dram_tensor('cov', [BL, T], f32, kind='ExternalOutput').ap()

    with tile.TileContext(nc) as tc:
        with tc.tile_pool(name='const', bufs=1) as cpool, \
             tc.tile_pool(name='ptr', bufs=3, space='PSUM') as p_tr, \
             tc.tile_pool(name='ph', bufs=2, space='PSUM') as p_h, \
             tc.tile_pool(name='psc', bufs=1, space='PSUM') as p_sc, \
             tc.tile_pool(name='pctx', bufs=1, space='PSUM') as p_ctx:

            # ---- persistent constants ----
            ws_sb = cpool.tile([P, KO, UNITS], f32r, tag='ws')
            nc.gpsimd.dma_start(ws_sb[:], ws_d.rearrange("(ko kp) u -> kp ko u", kp=P))
            wc_sb = cpool.tile([1, UNITS], f32r, tag='wc')
            nc.gpsimd.dma_start(wc_sb[:], wc_d[:, :])
            vw_sb = cpool.tile([P, UO], f32r, tag='vw')
            nc.gpsimd.dma_start(vw_sb[:], vw_d.rearrange("(o p) -> p o", p=P))
            vb_sb = cpool.tile([1, 1], f32, tag='vb')
            nc.sync.dma_start(vb_sb[:], vb_d[:, :])
            id_r = cpool.tile([P, P], f32r, tag='idr')
            ones1 = cpool.tile([1, 1], f32, tag='one')
            nc.vector.memset(ones1[:], 1.0)
            qbias = cpool.tile([P, UO, BL], f32, tag='qb')

            # ---- prologue (scoped pool: space reclaimed before the main loop) ----
            with tc.tile_pool(name='prolog', bufs=1) as ppool:
                wh_sb = ppool.tile([P, KO, UNITS], f32, tag='wh')
                nc.sync.dma_start(wh_sb[:], wh_d.rearrange("(ko kp) u -> kp ko u", kp=P))

                bias3 = ppool.tile([P, UO], f32, tag='b3')
                btmp1 = ppool.tile([P, UO], f32, tag='bt1')
                btmp2 = ppool.tile([P, UO], f32, tag='bt2')
                nc.sync.dma_start(bias3[:], bh_d.rearrange("(o p) -> p o", p=P))
                nc.sync.dma_start(btmp1[:], bs_d.rearrange("(o p) -> p o", p=P))
                nc.sync.dma_start(btmp2[:], bc_d.rearrange("(o p) -> p o", p=P))
                nc.vector.tensor_add(bias3[:], bias3[:], btmp1[:])
                nc.vector.tensor_add(bias3[:], bias3[:], btmp2[:])

                id_s = ppool.tile([P, P], f32, tag='id')
                make_identity(nc, id_s[:])
                nc.vector.tensor_copy(id_r[:], id_s[:])

                # query -> qT [dq_p, dq_o, BL] via PE transposes
                q_nat = ppool.tile([BL, DQ], f32, tag='qn')
                nc.sync.dma_start(q_nat[:], q_d[:, :])
                qT = ppool.tile([P, KO, BL], f32, tag='qT')
                for ko in range(KO):
                    pq = p_tr.tile([P, BL], f32, tag='tr')
                    nc.tensor.transpose(pq[:], q_nat[:, ko * P:(ko + 1) * P], id_s[0:BL, 0:BL])
                    nc.vector.tensor_copy(qT[:, ko, :], pq[:])

                # qbias[u, uo, b] = (query @ Wh)[b, u] + bh[u] + bs[u] + bc[u]
                for uo in range(UO):
                    pqp = p_tr.tile([P, BL], f32, tag='tr')
                    for ko in range(KO):
                        nc.tensor.matmul(pqp[:], wh_sb[:, ko, uo * P:(uo + 1) * P], qT[:, ko, :],
                                         start=(ko == 0), stop=(ko == KO - 1))
                    nc.vector.tensor_tensor(qbias[:, uo, :], pqp[:],
                                            bias3[:, uo:uo + 1].to_broadcast([P, BL]), Add)

            # ---- main loop over examples and t-chunks ----
            with tc.tile_pool(name='val', bufs=4) as vpool, \
                 tc.tile_pool(name='vt', bufs=2) as vtpool, \
                 tc.tile_pool(name='th', bufs=3) as hpool, \
                 tc.tile_pool(name='small', bufs=2) as spool:

                def emit_ctx(ch, vn, exp_b, pctx):
                    # transpose exp row into [t_p, tt], then accumulate
                    # pctx[0, d] += sum_t exp[t] * values[t, d]
                    petr = p_tr.tile([P, TT], f32, tag='tr')
                    for tt in range(TT):
                        nc.tensor.matmul(petr[:, tt:tt + 1],
                                         exp_b[0:1, (ch * TT + tt) * P:(ch * TT + tt + 1) * P],
                                         ones1[:], is_transpose=True,
                                         start=(tt == 0), stop=(tt == TT - 1),
                                         skip_group_check=True)
                    eT = spool.tile([P, TT], f32r, tag='eT')
                    nc.vector.tensor_copy(eT[:], petr[:])
                    for do in range(DO):
                        for tt in range(TT):
                            nc.tensor.matmul(pctx[0:1, do * 512:(do + 1) * 512],
                                             eT[:, tt:tt + 1], vn[:, tt, do * 512:(do + 1) * 512],
                                             start=(ch == 0 and tt == 0),
                                             stop=(ch == NCH - 1 and tt == TT - 1),
                                             skip_group_check=True)

                # global chunk-job list so vn DMAs prefetch 2 jobs ahead,
                # across example boundaries
                jobs = [(b, ch) for b in range(BL) for ch in range(NCH)]
                vn_tiles = {}

                def emit_vn_dma(j):
                    if j >= len(jobs):
                        return
                    jb, jch = jobs[j]
                    vn = vpool.tile([P, TT, DV], f32r, tag='vn')
                    nc.gpsimd.dma_start(
                        vn[:], v_d[jb].rearrange("(tt tp) d -> tp tt d", tp=P)[:, jch * TT:(jch + 1) * TT, :])
                    vn_tiles[j] = vn

                def emit_transpose_group(vn, vt, ko):
                    # 4 PE transposes [128,128] into one PSUM bank + 1 copy out
                    ptr = p_tr.tile([P, TCH], f32r, tag='tr')
                    for tt in range(TT):
                        nc.tensor.matmul(ptr[:, tt * P:(tt + 1) * P],
                                         vn[:, tt, ko * P:(ko + 1) * P], id_r[:],
                                         is_transpose=True,
                                         start=(tt == 0), stop=(tt == TT - 1),
                                         skip_group_check=True)
                    nc.vector.tensor_copy(vt[:, ko, :], ptr[:].bitcast(f32))

                emit_vn_dma(0)
                emit_vn_dma(1)
                for b in range(BL):
                    j0 = b * NCH
                    exp_b = spool.tile([1, T], f32, tag='exp')
                    acc_b = spool.tile([1, NCH], f32, tag='acc')
                    cov_f = spool.tile([1, T], f32, tag='cvf')
                    nc.sync.dma_start(cov_f[:], c_d[b:b + 1, :])
                    pctx = p_ctx.tile([1, DV], f32, tag='ctx')

                    # chunk 0: standalone transpose block (nothing to hide under)
                    vts = {}
                    vts[0] = vtpool.tile([P, KO, TCH], f32r, tag='vt', name=f'vt_{b}_0')
                    for ko in range(KO):
                        emit_transpose_group(vn_tiles[j0], vts[0], ko)

                    for ch in range(NCH):
                        j = j0 + ch
                        emit_vn_dma(j + 2)
                        cvr = spool.tile([1, TCH], f32r, tag='cvr')
                        nc.gpsimd.dma_start(cvr[:], c_d[b:b + 1, ch * TCH:(ch + 1) * TCH])
                        if ch + 1 < NCH:
                            vts[ch + 1] = vtpool.tile([P, KO, TCH], f32r, tag='vt', name=f'vt_{b}_{ch+1}')

                        # H^T tiles + tanh + score contraction over u; the NEXT
                        # chunk's transposes are interleaved so their weight
                        # loads hide under the long main matmuls
                        vt = vts[ch]
                        psc = p_sc.tile([1, TCH], f32, tag='sc')
                        ths = []
                        for uo in range(UO):
                            if ch + 1 < NCH:
                                emit_transpose_group(vn_tiles[j + 1], vts[ch + 1], uo)
                            ph = p_h.tile([P, TCH], f32, tag='h')
                            nc.tensor.matmul(ph[:], wc_sb[0:1, uo * P:(uo + 1) * P], cvr[:],
                                             start=True, stop=False)
                            for ko in range(KO):
                                nc.tensor.matmul(ph[:], ws_sb[:, ko, uo * P:(uo + 1) * P], vt[:, ko, :],
                                                 start=False, stop=(ko == KO - 1))
                            th = hpool.tile([P, TCH], f32r, tag='th')
                            nc.scalar.activation(th[:], ph[:], Tanh, bias=qbias[:, uo, b:b + 1])
                            ths.append(th)
                            if uo > 0:
                                nc.tensor.matmul(psc[:], vw_sb[:, uo - 1:uo], ths[uo - 1][:],
                                                 start=(uo == 1), stop=False,
                                                 skip_group_check=True)
                        nc.tensor.matmul(psc[:], vw_sb[:, UO - 1:UO], ths[UO - 1][:],
                                         start=False, stop=True, skip_group_check=True)

                        # exp (+ Vb bias) with free per-chunk sum
                        nc.scalar.activation(exp_b[0:1, ch * TCH:(ch + 1) * TCH], psc[:], Exp,
                                             bias=vb_sb[:], accum_out=acc_b[0:1, ch:ch + 1])

                        # context contribution of the PREVIOUS chunk (deferred
                        # one chunk so the PE rides ahead of ACT's exp)
                        if ch > 0:
                            emit_ctx(ch - 1, vn_tiles[j - 1], exp_b, pctx)
                    emit_ctx(NCH - 1, vn_tiles[j0 + NCH - 1], exp_b, pctx)

                    # ---- per-example epilogue ----
                    sum_b = spool.tile([1, 1], f32, tag='sum')
                    nc.vector.tensor_reduce(sum_b[:], acc_b[:], X, Add)
                    rec_b = spool.tile([1, 1], f32, tag='rec')
                    nc.vector.reciprocal(rec_b[:], sum_b[:])
                    ctx_sb = spool.tile([1, DV], f32, tag='ctxo')
                    nc.vector.tensor_scalar_mul(ctx_sb[:], pctx[:], rec_b[:])
                    nc.sync.dma_start(ctx_d[b:b + 1, :], ctx_sb[:])
                    nc.vector.tensor_scalar_mul(exp_b[:], exp_b[:], rec_b[:])
                    nc.sync.dma_start(attn_d[b:b + 1, :], exp_b[:])
                    nc.vector.tensor_add(cov_f[:], cov_f[:], exp_b[:])
                    nc.sync.dma_start(cov_d[b:b + 1, :], cov_f[:])

    nc.compile()
    return nc


def kernel(query, values, prev_coverage, Wh, bh, Ws, bs, Wc, bc, Vw, Vb):
    from concourse.bass_utils import run_bass_kernel_spmd

    if 'nc' not in _cache:
        _cache['nc'] = _build()
    nc = _cache['nc']

    query = np.ascontiguousarray(np.asarray(query, dtype=np.float32))
    values = np.ascontiguousarray(np.asarray(values, dtype=np.float32))
    cov = np.ascontiguousarray(
        np.asarray(prev_coverage, dtype=np.float32).reshape(B, T))
    rep = {
        'Wh': np.ascontiguousarray(np.asarray(Wh, dtype=np.float32)),
        'bh': np.ascontiguousarray(np.asarray(bh, dtype=np.float32)),
        'Ws': np.ascontiguousarray(np.asarray(Ws, dtype=np.float32)),
        'bs': np.ascontiguousarray(np.asarray(bs, dtype=np.float32)),
        'Wc': np.ascontiguousarray(np.asarray(Wc, dtype=np.float32).reshape(1, UNITS)),
        'bc': np.ascontiguousarray(np.asarray(bc, dtype=np.float32)),
        'Vw': np.ascontiguousarray(np.asarray(Vw, dtype=np.float32).reshape(UNITS)),
        'Vb': np.ascontiguousarray(np.asarray(Vb, dtype=np.float32).reshape(1, 1)),
    }
    in_maps = []
    for c in range(NC_):
        s = slice(c * BL, (c + 1) * BL)
        in_maps.append({
            'query': query[s],
            'values': values[s],
            'prev_coverage': cov[s],
            **rep,
        })

    res = run_bass_kernel_spmd(nc, in_maps, core_ids=list(range(NC_)))
    context = np.concatenate([res.results[c]['ctx'] for c in range(NC_)], axis=0)
    attn = np.concatenate([res.results[c]['attn'] for c in range(NC_)], axis=0)
    coverage = np.concatenate([res.results[c]['cov'] for c in range(NC_)], axis=0)
    return context, attn, coverage.reshape(B, T, 1)


# revision 10
# speedup vs baseline: 1.0340x; 1.0199x over previous
"""Bahdanau attention with coverage — distributed Bass kernel for 8 TRN2 NeuronCores.

Data-parallel over batch: each core handles B/8 = 4 examples end-to-end; the
small projection weights are replicated. No inter-core collectives needed.

Per example (T=2048, DQ=DV=UNITS=1024):
  H[t,u]   = values[t,:] @ Ws[:,u] + cov[t]*Wc[u] + (query @ Wh + bh + bs + bc)[u]
  score[t] = tanh(H[t,:]) @ Vw + Vb
  attn     = softmax(score) (no max-shift needed: |score| stays tiny)
  coverage = attn + prev_coverage
  context  = attn @ values

On-chip layout: H is computed transposed ([u_partitions, t_free]) so the
query/bias terms become per-partition activation biases, the coverage term a
rank-1 (K=1) PSUM-accumulated matmul, and score a PE contraction over u.
values tiles are PE-transposed on the fly for the main matmul; the same native
tiles feed the context matmul, with unnormalized exp weights accumulated in
PSUM across chunks and one rescale at the end. Matmuls run in float32r
(~TF32 precision, 4x the fp32 PE throughput at moving dim >= 256).

Scheduling notes:
  - matmul start=True zeroes the whole 2KB PSUM bank ("zero region"), so only
    the first matmul/transpose into a bank sets it; later ones accumulate.
  - the score matmul of u-tile uo is emitted after the main matmuls of uo+1,
    and the context matmuls of chunk ch after the transposes of chunk ch+1,
    so the PE never stalls waiting on ACT.
"""

import numpy as np

B, T, DQ, DV, UNITS = 32, 2048, 1024, 1024, 1024
NC_ = 8
BL = B // NC_          # 4 examples per core
P = 128
TCH = 512              # t-chunk size (main-MM moving dim)
NCH = T // TCH         # 4 chunks per example
TT = TCH // P          # 4 t-tiles per chunk
KO = DV // P           # 8 contraction chunks
UO = UNITS // P        # 8 u tiles
DO = DV // 512         # 2 dv chunks for context MM

_cache = {}


def _build():
    import concourse.mybir as mybir
    from concourse import bacc
    import concourse.tile as tile
    from concourse.masks import make_identity

    f32 = mybir.dt.float32
    f32r = mybir.dt.float32r
    Tanh = mybir.ActivationFunctionType.Tanh
    Exp = mybir.ActivationFunctionType.Exp
    Add = mybir.AluOpType.add
    X = mybir.AxisListType.X

    nc = bacc.Bacc('TRN2', target_bir_lowering=False, debug=False, num_devices=NC_)
    q_d = nc.dram_tensor('query', [BL, DQ], f32, kind='ExternalInput').ap()
    v_d = nc.dram_tensor('values', [BL, T, DV], f32, kind='ExternalInput').ap()
    c_d = nc.dram_tensor('prev_coverage', [BL, T], f32, kind='ExternalInput').ap()
    wh_d = nc.dram_tensor('Wh', [DQ, UNITS], f32, kind='ExternalInput').ap()
    bh_d = nc.dram_tensor('bh', [UNITS], f32, kind='ExternalInput').ap()
    ws_d = nc.dram_tensor('Ws', [DV, UNITS], f32, kind='ExternalInput').ap()
    bs_d = nc.dram_tensor('bs', [UNITS], f32, kind='ExternalInput').ap()
    wc_d = nc.dram_tensor('Wc', [1, UNITS], f32, kind='ExternalInput').ap()
    bc_d = nc.dram_tensor('bc', [UNITS], f32, kind='ExternalInput').ap()
    vw_d = nc.dram_tensor('Vw', [UNITS], f32, kind='ExternalInput').ap()
    vb_d = nc.dram_tensor('Vb', [1, 1], f32, kind='ExternalInput').ap()
    ctx_d = nc.dram_tensor('ctx', [BL, DV], f32, kind='ExternalOutput').ap()
    attn_d = nc.dram_tensor('attn', [BL, T], f32, kind='ExternalOutput').ap()
    cov_d = nc.dram_tensor('cov', [BL, T], f32, kind='ExternalOutput').ap()

    with tile.TileContext(nc) as tc:
        with tc.tile_pool(name='const', bufs=1) as cpool, \
             tc.tile_pool(name='ptr', bufs=3, space='PSUM') as p_tr, \
             tc.tile_pool(name='ph', bufs=2, space='PSUM') as p_h, \
             tc.tile_pool(name='psc', bufs=1, space='PSUM') as p_sc, \
             tc.tile_pool(name='pctx', bufs=1, space='PSUM') as p_ctx:

            # ---- persistent constants ----
            ws_sb = cpool.tile([P, KO, UNITS], f32r, tag='ws')
            nc.gpsimd.dma_start(ws_sb[:], ws_d.rearrange("(ko kp) u -> kp ko u", kp=P))
            wc_sb = cpool.tile([1, UNITS], f32r, tag='wc')
            nc.gpsimd.dma_start(wc_sb[:], wc_d[:, :])
            vw_sb = cpool.tile([P, UO], f32r, tag='vw')
            nc.gpsimd.dma_start(vw_sb[:], vw_d.rearrange("(o p) -> p o", p=P))
            vb_sb = cpool.tile([1, 1], f32, tag='vb')
            nc.sync.dma_start(vb_sb[:], vb_d[:, :])
            id_r = cpool.tile([P, P], f32r, tag='idr')
            ones1 = cpool.tile([1, 1], f32, tag='one')
            nc.vector.memset(ones1[:], 1.0)
            qbias = cpool.tile([P, UO, BL], f32, tag='qb')

            # ---- prologue (scoped pool: space reclaimed before the main loop) ----
            with tc.tile_pool(name='prolog', bufs=1) as ppool:
                wh_sb = ppool.tile([P, KO, UNITS], f32, tag='wh')
                nc.sync.dma_start(wh_sb[:], wh_d.rearrange("(ko kp) u -> kp ko u", kp=P))

                bias3 = ppool.tile([P, UO], f32, tag='b3')
                btmp1 = ppool.tile([P, UO], f32, tag='bt1')
                btmp2 = ppool.tile([P, UO], f32, tag='bt2')
                nc.sync.dma_start(bias3[:], bh_d.rearrange("(o p) -> p o", p=P))
                nc.sync.dma_start(btmp1[:], bs_d.rearrange("(o p) -> p o", p=P))
                nc.sync.dma_start(btmp2[:], bc_d.rearrange("(o p) -> p o", p=P))
                nc.vector.tensor_add(bias3[:], bias3[:], btmp1[:])
                nc.vector.tensor_add(bias3[:], bias3[:], btmp2[:])

                id_s = ppool.tile([P, P], f32, tag='id')
                make_identity(nc, id_s[:])
                nc.vector.tensor_copy(id_r[:], id_s[:])

                # query -> qT [dq_p, dq_o, BL] via PE transposes
                q_nat = ppool.tile([BL, DQ], f32, tag='qn')
                nc.sync.dma_start(q_nat[:], q_d[:, :])
                qT = ppool.tile([P, KO, BL], f32, tag='qT')
                for ko in range(KO):
                    pq = p_tr.tile([P, BL], f32, tag='tr')
                    nc.tensor.transpose(pq[:], q_nat[:, ko * P:(ko + 1) * P], id_s[0:BL, 0:BL])
                    nc.vector.tensor_copy(qT[:, ko, :], pq[:])

                # qbias[u, uo, b] = (query @ Wh)[b, u] + bh[u] + bs[u] + bc[u]
                for uo in range(UO):
                    pqp = p_tr.tile([P, BL], f32, tag='tr')
                    for ko in range(KO):
                        nc.tensor.matmul(pqp[:], wh_sb[:, ko, uo * P:(uo + 1) * P], qT[:, ko, :],
                                         start=(ko == 0), stop=(ko == KO - 1))
                    nc.vector.tensor_tensor(qbias[:, uo, :], pqp[:],
                                            bias3[:, uo:uo + 1].to_broadcast([P, BL]), Add)

            # ---- main loop over examples and t-chunks ----
            with tc.tile_pool(name='val', bufs=4) as vpool, \
                 tc.tile_pool(name='vt', bufs=2) as vtpool, \
                 tc.tile_pool(name='th', bufs=3) as hpool, \
                 tc.tile_pool(name='small', bufs=2) as spool:

                def emit_ctx(ch, vn, exp_b, pctx):
                    # transpose exp row into [t_p, tt], then accumulate
                    # pctx[0, d] += sum_t exp[t] * values[t, d]
                    petr = p_tr.tile([P, TT], f32, tag='tr')
                    for tt in range(TT):
                        nc.tensor.matmul(petr[:, tt:tt + 1],
                                         exp_b[0:1, (ch * TT + tt) * P:(ch * TT + tt + 1) * P],
                                         ones1[:], is_transpose=True,
                                         start=(tt == 0), stop=(tt == TT - 1),
                                         skip_group_check=True)
                    eT = spool.tile([P, TT], f32r, tag='eT')
                    nc.vector.tensor_copy(eT[:], petr[:])
                    for do in range(DO):
                        for tt in range(TT):
                            nc.tensor.matmul(pctx[0:1, do * 512:(do + 1) * 512],
                                             eT[:, tt:tt + 1], vn[:, tt, do * 512:(do + 1) * 512],
                                             start=(ch == 0 and tt == 0),
                                             stop=(ch == NCH - 1 and tt == TT - 1),
                                             skip_group_check=True)

                # global chunk-job list so vn DMAs prefetch 2 jobs ahead,
                # across example boundaries
                jobs = [(b, ch) for b in range(BL) for ch in range(NCH)]
                vn_tiles = {}

                def emit_vn_dma(j):
                    if j >= len(jobs):
                        return
                    jb, jch = jobs[j]
                    vn = vpool.tile([P, TT, DV], f32r, tag='vn')
                    nc.gpsimd.dma_start(
                        vn[:], v_d[jb].rearrange("(tt tp) d -> tp tt d", tp=P)[:, jch * TT:(jch + 1) * TT, :])
                    vn_tiles[j] = vn

                def emit_transpose_group(vn, vt, ko):
                    # 4 PE transposes [128,128] into one PSUM bank + 1 copy out
                    ptr = p_tr.tile([P, TCH], f32r, tag='tr')
                    for tt in range(TT):
                        nc.tensor.matmul(ptr[:, tt * P:(tt + 1) * P],
                                         vn[:, tt, ko * P:(ko + 1) * P], id_r[:],
                                         is_transpose=True,
                                         start=(tt == 0), stop=(tt == TT - 1),
                                         skip_group_check=True)
                    nc.vector.tensor_copy(vt[:, ko, :], ptr[:].bitcast(f32))

                st = {}        # per-example tiles
                vt_tiles = {}  # j -> transposed-values tile

                def make_state(sb_):
                    exp_b = spool.tile([1, T], f32, tag='exp', name=f'exp_{sb_}')
                    acc_b = spool.tile([1, NCH], f32, tag='acc', name=f'acc_{sb_}')
                    cov_f = spool.tile([1, T], f32, tag='cvf', name=f'cvf_{sb_}')
                    nc.sync.dma_start(cov_f[:], c_d[sb_:sb_ + 1, :])
                    pctx = p_ctx.tile([1, DV], f32, tag='ctx', name=f'pctx_{sb_}')
                    st[sb_] = dict(exp=exp_b, acc=acc_b, cvf=cov_f, pctx=pctx)

                def emit_epilogue(eb):
                    s = st[eb]
                    sum_b = spool.tile([1, 1], f32, tag='sum', name=f'sum_{eb}')
                    nc.vector.tensor_reduce(sum_b[:], s['acc'][:], X, Add)
                    rec_b = spool.tile([1, 1], f32, tag='rec', name=f'rec_{eb}')
                    nc.vector.reciprocal(rec_b[:], sum_b[:])
                    ctx_sb = spool.tile([1, DV], f32, tag='ctxo', name=f'ctxo_{eb}')
                    nc.vector.tensor_scalar_mul(ctx_sb[:], s['pctx'][:], rec_b[:])
                    nc.sync.dma_start(ctx_d[eb:eb + 1, :], ctx_sb[:])
                    nc.vector.tensor_scalar_mul(s['exp'][:], s['exp'][:], rec_b[:])
                    nc.sync.dma_start(attn_d[eb:eb + 1, :], s['exp'][:])
                    nc.vector.tensor_add(s['cvf'][:], s['cvf'][:], s['exp'][:])
                    nc.sync.dma_start(cov_d[eb:eb + 1, :], s['cvf'][:])

                emit_vn_dma(0)
                emit_vn_dma(1)
                make_state(0)

                # job 0's transposes have nothing to hide under (pipeline start)
                vt_tiles[0] = vtpool.tile([P, KO, TCH], f32r, tag='vt', name='vt_0')
                for ko in range(KO):
                    emit_transpose_group(vn_tiles[0], vt_tiles[0], ko)

                for j, (b, ch) in enumerate(jobs):
                    s = st[b]
                    emit_vn_dma(j + 2)
                    cvr = spool.tile([1, TCH], f32r, tag='cvr', name=f'cvr_{j}')
                    nc.gpsimd.dma_start(cvr[:], c_d[b:b + 1, ch * TCH:(ch + 1) * TCH])
                    nj = j + 1 if j + 1 < len(jobs) else None
                    if nj is not None:
                        nb = jobs[nj][0]
                        if nb not in st:
                            make_state(nb)
                        vt_tiles[nj] = vtpool.tile([P, KO, TCH], f32r, tag='vt',
                                                   name=f'vt_{nj}')

                    # per u-tile: next job's transposes (weight loads hide under
                    # the long main matmuls), rank-1 coverage term + 8 main MMs,
                    # tanh on ACT, score MM one u-tile behind. The previous
                    # job's exp-transpose + context MMs slot in at uo==4.
                    vt = vt_tiles[j]
                    psc = p_sc.tile([1, TCH], f32, tag='sc', name=f'psc_{j}')
                    ths = []
                    for uo in range(UO):
                        if nj is not None:
                            emit_transpose_group(vn_tiles[nj], vt_tiles[nj], uo)
                        if uo == 4 and j > 0:
                            pb, pch = jobs[j - 1]
                            emit_ctx(pch, vn_tiles[j - 1], st[pb]['exp'], st[pb]['pctx'])
                            if pch == NCH - 1:
                                emit_epilogue(pb)
                        ph = p_h.tile([P, TCH], f32, tag='h')
                        nc.tensor.matmul(ph[:], wc_sb[0:1, uo * P:(uo + 1) * P], cvr[:],
                                         start=True, stop=False)
                        for ko in range(KO):
                            nc.tensor.matmul(ph[:], ws_sb[:, ko, uo * P:(uo + 1) * P], vt[:, ko, :],
                                             start=False, stop=(ko == KO - 1))
                        th = hpool.tile([P, TCH], f32r, tag='th')
                        nc.scalar.activation(th[:], ph[:], Tanh, bias=qbias[:, uo, b:b + 1])
                        ths.append(th)
                        if uo > 0:
                            nc.tensor.matmul(psc[:], vw_sb[:, uo - 1:uo], ths[uo - 1][:],
                                             start=(uo == 1), stop=False,
                                             skip_group_check=True)
                    nc.tensor.matmul(psc[:], vw_sb[:, UO - 1:UO], ths[UO - 1][:],
                                     start=False, stop=True, skip_group_check=True)

                    # exp (+ Vb bias) with free per-chunk sum
                    nc.scalar.activation(s['exp'][0:1, ch * TCH:(ch + 1) * TCH], psc[:], Exp,
                                         bias=vb_sb[:], accum_out=s['acc'][0:1, ch:ch + 1])

                jl = len(jobs) - 1
                emit_ctx(jobs[jl][1], vn_tiles[jl], st[BL - 1]['exp'], st[BL - 1]['pctx'])
                emit_epilogue(BL - 1)

    nc.compile()
    return nc


def kernel(query, values, prev_coverage, Wh, bh, Ws, bs, Wc, bc, Vw, Vb):
    from concourse.bass_utils import run_bass_kernel_spmd

    if 'nc' not in _cache:
        _cache['nc'] = _build()
    nc = _cache['nc']

    query = np.ascontiguousarray(np.asarray(query, dtype=np.float32))
    values = np.ascontiguousarray(np.asarray(values, dtype=np.float32))
    cov = np.ascontiguousarray(
        np.asarray(prev_coverage, dtype=np.float32).reshape(B, T))
    rep = {
        'Wh': np.ascontiguousarray(np.asarray(Wh, dtype=np.float32)),
        'bh': np.ascontiguousarray(np.asarray(bh, dtype=np.float32)),
        'Ws': np.ascontiguousarray(np.asarray(Ws, dtype=np.float32)),
        'bs': np.ascontiguousarray(np.asarray(bs, dtype=np.float32)),
        'Wc': np.ascontiguousarray(np.asarray(Wc, dtype=np.float32).reshape(1, UNITS)),
        'bc': np.ascontiguousarray(np.asarray(bc, dtype=np.float32)),
        'Vw': np.ascontiguousarray(np.asarray(Vw, dtype=np.float32).reshape(UNITS)),
        'Vb': np.ascontiguousarray(np.asarray(Vb, dtype=np.float32).reshape(1, 1)),
    }
    in_maps = []
    for c in range(NC_):
        s = slice(c * BL, (c + 1) * BL)
        in_maps.append({
            'query': query[s],
            'values': values[s],
            'prev_coverage': cov[s],
            **rep,
        })

    res = run_bass_kernel_spmd(nc, in_maps, core_ids=list(range(NC_)))
    context = np.concatenate([res.results[c]['ctx'] for c in range(NC_)], axis=0)
    attn = np.concatenate([res.results[c]['attn'] for c in range(NC_)], axis=0)
    coverage = np.concatenate([res.results[c]['cov'] for c in range(NC_)], axis=0)
    return context, attn, coverage.reshape(B, T, 1)
